# revision 41
# baseline (speedup 1.0000x reference)
"""Mixer (token-mix + channel-mix MLP) kernel for 8 TRN2 NeuronCores.

v3: pair-packed [batch, feature] design (expert-parallel over group axes).
  All elementwise/activation work runs on the full 128 partitions by packing
  two groups per op: phase 1 packs channels (ci, ci+4) of each 8-channel
  chunk as partitions (b, b+64); phase 2 packs patches (nl, nl+16).
  Per pair: LN1 via bn_stats/bn_aggr (one per iteration, pipelined),
  xn = x*rstd+nmr as one DVE tensor_scalar; ONE matmul per 128-block
  transposes both packed groups against a shared identity; fc1 is
  weight-stationary (bias-free when biases are zero); gelu is a single
  full-width activation; fc2 is weight-moving (256-wide moving operand)
  with the pair's outputs landing in one PSUM tile via tile_position
  column offsets; u = x + tok is one DVE scalar_tensor_tensor with fused
  sum; sum(u^2) one scalar Square with accum; yn = u*rstd2+nmr2 one
  activation. u and yn ship bf16 over 4 chunked AllToAlls that overlap the
  phase-1 tail. Phase 2 needs no normalization; y = u + ch per pair, output
  chunked. Weights stream as per-pair DMAs ([128, 2, 4, 256] blocks).
"""
import sys
import numpy as np

sys.path.insert(0, "/opt/trn_rl_repo")

import ml_dtypes
import concourse.bass as bass
import concourse.bacc as bacc
import concourse.tile as tile
from concourse import mybir
from concourse.bass_utils import run_bass_kernel_spmd

F32 = mybir.dt.float32
BF16 = mybir.dt.bfloat16
NCORE = 8
B, C, N = 64, 256, 256
CL = C // NCORE   # 32 local channels (phase 1)
NL = N // NCORE   # 32 local patches (phase 2)
EPS = 1e-5
GELU = mybir.ActivationFunctionType.Gelu
IDENT = mybir.ActivationFunctionType.Identity
SQRT = mybir.ActivationFunctionType.Sqrt
SQUARE = mybir.ActivationFunctionType.Square
MUL = mybir.AluOpType.mult
ADD = mybir.AluOpType.add

CH = 8                 # channels per stats/collective chunk
NCHUNK = CL // CH      # 4 chunks
PPC = 4                # pairs per chunk
NPAIR = CL // 2        # 16 channel pairs / 16 patch pairs
BLK = CH * NL          # 256 elems per (dest, tensor) block row


def build_program(gelu_func=GELU, mmdt=BF16, ws=1.0, skip_b1=True,
                  skip_b2=True, skip_bc1=True, skip_bc2=True, dbg=False):
    nc = bacc.Bacc("TRN2", target_bir_lowering=False, debug=False,
                   enable_asserts=True, num_devices=NCORE)
    wsi = 1.0 / ws

    x_in = nc.dram_tensor("x_sh", [2, B, PPC * NCHUNK, N], F32,
                          kind="ExternalInput")
    wt_in = nc.dram_tensor("wt", [NPAIR, 128, 2, 4, N], mmdt,
                           kind="ExternalInput")
    ct_in = nc.dram_tensor("ct", [NPAIR, 128, 2, 4, C], mmdt,
                           kind="ExternalInput")
    id128_in = nc.dram_tensor("id128", [128, 128], BF16,
                              kind="ExternalInput")
    if not skip_b1:
        b1p_in = nc.dram_tensor("b1p", [4, NPAIR, 128], BF16,
                                kind="ExternalInput")
    if not (skip_b1 and skip_bc1):
        msel_in = nc.dram_tensor("msel", [4, 256], BF16,
                                 kind="ExternalInput")
    if not skip_bc1:
        bc1p_in = nc.dram_tensor("bc1p", [4, NPAIR, 128], BF16,
                                 kind="ExternalInput")
    if not skip_b2:
        b2r_in = nc.dram_tensor("b2r", [CL, N], BF16, kind="ExternalInput")
    if not skip_bc2:
        bc2r_in = nc.dram_tensor("bc2r", [NL, C], BF16, kind="ExternalInput")

    ybuf = nc.dram_tensor("ybuf", [B, NL, C], BF16, kind="ExternalOutput")
    if dbg:
        xn_dbg = nc.dram_tensor("xn_dbg", [128, N], BF16,
                                kind="ExternalOutput")
        z_dbg = nc.dram_tensor("z_dbg", [128, 2, 128], BF16,
                               kind="ExternalOutput")
        rstd_dbg = nc.dram_tensor("rstd_dbg", [128, 2 * NPAIR], F32,
                                  kind="ExternalOutput")
        u_dbg = nc.dram_tensor("u_dbg", [128, NCORE, 2, PPC, 16], BF16,
                               kind="ExternalOutput")
        yn2_dbg = nc.dram_tensor("yn2_dbg", [128, C, 16], BF16,
                                 kind="ExternalOutput")
        u2_dbg = nc.dram_tensor("u2_dbg", [128, C, 16], BF16,
                                kind="ExternalOutput")

    with tile.TileContext(nc) as tc:
        with tc.tile_pool(name="const", bufs=1) as const, \
             tc.tile_pool(name="wpool", bufs=6) as wpool, \
             tc.tile_pool(name="act", bufs=3) as act, \
             tc.tile_pool(name="scr", bufs=2) as scr, \
             tc.tile_pool(name="dram", bufs=1, space="DRAM") as dram, \
             tc.tile_pool(name="ps", bufs=2, space="PSUM") as ps:

            # block: [dest, {yn,u}, nl-half, b, chunk-in-group, ci, nlh]
            # three collectives over chunk groups: {0}, {1,2}, {3} — the
            # first fires early enough to hide under phase-1 compute
            CCG = [[0], [1, 2], [3]]
            grp_of = {k: g for g, ks in enumerate(CCG) for k in ks}
            sendb = [dram.tile([NCORE, 2, 2, B, len(ks), CH, 16], BF16,
                               name=f"snd{g}") for g, ks in enumerate(CCG)]
            recvb = [dram.tile([NCORE, 2, 2, B, len(ks), CH, 16], BF16,
                               name=f"rcv{g}") for g, ks in enumerate(CCG)]

            id128 = const.tile([128, 128], BF16)
            nc.sync.dma_start(out=id128[:], in_=id128_in[:])
            if not skip_b1:
                b1p = const.tile([4, NPAIR, 128], BF16)
                nc.sync.dma_start(out=b1p[:], in_=b1p_in[:])
            if not (skip_b1 and skip_bc1):
                msel = const.tile([4, 256], BF16)
                nc.sync.dma_start(out=msel[:], in_=msel_in[:])
            if not skip_bc1:
                bc1p = const.tile([4, NPAIR, 128], BF16)
                nc.sync.dma_start(out=bc1p[:], in_=bc1p_in[:])
            if not skip_b2:
                b2r = const.tile([CL, N], BF16)
                nc.sync.dma_start(out=b2r[:], in_=b2r_in[:])
                ones1 = const.tile([1, 64], BF16)
                nc.vector.memset(ones1[:], 1.0)
            if not skip_bc2:
                bc2r = const.tile([NL, C], BF16)
                nc.sync.dma_start(out=bc2r[:], in_=bc2r_in[:])
                ones1c = const.tile([1, 64], BF16)
                nc.vector.memset(ones1c[:], 1.0)
            eps128 = const.tile([128, 1], F32)
            nc.vector.memset(eps128[:], EPS)

            # x pair-packed: partition (par, b); par=ci//4 within chunk
            x_sb = [const.tile([128, PPC, N], F32, name=f"x{k}")
                    for k in range(NCHUNK)]
            for k in range(NCHUNK):
                for par in range(2):
                    nc.sync.dma_start(
                        out=x_sb[k][par * 64:(par + 1) * 64, :, :],
                        in_=x_in[par, :, k * PPC:(k + 1) * PPC, :])

            # [128(par,b), dest, nl-half, pair, nlh]
            u_pk = [const.tile([128, NCORE, 2, PPC, 16], BF16,
                               name=f"u{k}") for k in range(NCHUNK)]
            yn_pk = [const.tile([128, NCORE, 2, PPC, 16], BF16,
                                name=f"yn{k}") for k in range(NCHUNK)]

            # LN stats, one column per pair P
            st2 = const.tile([128, NPAIR, 2], F32)   # bn_aggr (mean, var)
            rstd1 = const.tile([128, NPAIR], F32)
            nmr1 = const.tile([128, NPAIR], F32)
            s2 = const.tile([128, NPAIR], F32)
            s2q = const.tile([128, NPAIR], F32)
            mu2 = const.tile([128, NPAIR], F32)
            rstd2 = const.tile([128, NPAIR], F32)
            nmr2 = const.tile([128, NPAIR], F32)
            tv = const.tile([128, NPAIR], F32)
            ts_ = const.tile([128, NPAIR], F32)

            def bn1(P):
                k, p = divmod(P, PPC)
                st6 = scr.tile([128, 6], F32, tag="st6")
                nc.vector.bn_stats(out=st6[:], in_=x_sb[k][:, p, :])
                nc.vector.bn_aggr(out=st2[:, P, :], in_=st6[:])

            def ln1_batch(k):
                cs = slice(k * PPC, (k + 1) * PPC)
                nc.scalar.activation(out=ts_[:, cs], in_=st2[:, cs, 1],
                                     func=SQRT, bias=eps128[:], scale=1.0)
                nc.vector.reciprocal(out=rstd1[:, cs], in_=ts_[:, cs])
                nc.vector.scalar_tensor_tensor(
                    out=nmr1[:, cs], in0=st2[:, cs, 0], scalar=-1.0,
                    in1=rstd1[:, cs], op0=MUL, op1=MUL)

            def ln2_batch(k):
                cs = slice(k * PPC, (k + 1) * PPC)
                nc.vector.tensor_scalar_mul(out=mu2[:, cs], in0=s2[:, cs],
                                            scalar1=1.0 / N)
                nc.vector.tensor_scalar_mul(out=tv[:, cs], in0=s2q[:, cs],
                                            scalar1=1.0 / N)
                nc.vector.tensor_mul(out=ts_[:, cs], in0=mu2[:, cs],
                                     in1=mu2[:, cs])
                nc.vector.tensor_sub(out=tv[:, cs], in0=tv[:, cs],
                                     in1=ts_[:, cs])
                nc.scalar.activation(out=ts_[:, cs], in_=tv[:, cs],
                                     func=SQRT, bias=eps128[:], scale=1.0)
                nc.vector.reciprocal(out=rstd2[:, cs], in_=ts_[:, cs])
                nc.vector.scalar_tensor_tensor(
                    out=nmr2[:, cs], in0=mu2[:, cs], scalar=-1.0,
                    in1=rstd2[:, cs], op0=MUL, op1=MUL)

            def stage1_pre(P):
                """xn + pair transpose + z copy for pair P."""
                k, p = divmod(P, PPC)
                xn = act.tile([128, N], BF16, tag="xn")
                nc.vector.tensor_scalar(
                    out=xn[:], in0=x_sb[k][:, p, :],
                    scalar1=rstd1[:, P:P + 1], scalar2=nmr1[:, P:P + 1],
                    op0=MUL, op1=ADD)
                zxp = ps.tile([128, 2, 128], F32, tag="zxp")
                for blk in range(2):
                    nc.tensor.matmul(
                        zxp[:, blk, :],
                        xn[:, blk * 128:(blk + 1) * 128],
                        id128[:], start=True, stop=True)
                z_sb = act.tile([128, 2, 128], BF16, tag="z")
                nc.vector.tensor_copy(out=z_sb[:], in_=zxp[:])
                if dbg and P == 0:
                    nc.scalar.dma_start(out=xn_dbg[:], in_=xn[:])
                return z_sb

            def mix_pair(P, z_sb, wp, bp, skip_b):
                """fc1 + gelu + fc2 for both groups of pair P."""
                hpre = ps.tile([128, 2, 2, 64], F32, tag="hpre")
                if not skip_b:
                    nc.tensor.matmul(
                        hpre[:].rearrange("p a b c -> p (a b c)"),
                        bp[:, P, :], msel[:],
                        start=True, stop=False, skip_group_check=True)
                for par in range(2):
                    for mb in range(2):
                        for nb in range(2):
                            nc.tensor.matmul(
                                hpre[:, mb, par, :],
                                wp[:, par, nb, mb * 128:(mb + 1) * 128],
                                z_sb[:, nb, par * 64:(par + 1) * 64],
                                start=(skip_b and nb == 0), stop=(nb == 1),
                                skip_group_check=True)
                hs = act.tile([128, 2, 2, 64], BF16, tag="h")
                nc.scalar.activation(out=hs[:], in_=hpre[:], func=gelu_func,
                                     scale=wsi)

                tok = ps.tile([128, 256], F32, tag="tok")
                for par in range(2):
                    for mb in range(2):
                        nc.tensor.matmul(
                            tok[par * 64:(par + 1) * 64, :],
                            hs[:, mb, par, :],
                            wp[:, par, 2 + mb, :],
                            start=(mb == 0), stop=(mb == 1),
                            skip_group_check=True)
                return tok

            def stage1_post(P, tok):
                k, p = divmod(P, PPC)
                tok3 = tok.rearrange("q (d a n) -> q d a n", d=NCORE, a=2)
                x3 = x_sb[k][:, p, :].rearrange("q (d a n) -> q d a n",
                                                d=NCORE, a=2)
                nc.vector.scalar_tensor_tensor(
                    out=u_pk[k][:, :, :, p, :], in0=tok3, scalar=wsi,
                    in1=x3, op0=MUL, op1=ADD, accum_out=s2[:, P:P + 1])
                sqs = scr.tile([128, NCORE, 2, 16], BF16, tag="sqs")
                nc.scalar.activation(out=sqs[:], in_=u_pk[k][:, :, :, p, :],
                                     func=SQUARE,
                                     accum_out=s2q[:, P:P + 1])

            # ---------------- phase 1 ----------------
            for P in range(PPC):
                bn1(P)
            ln1_batch(0)
            z_cur = stage1_pre(0)
            for P in range(NPAIR):
                k, p = divmod(P, PPC)
                wp = wpool.tile([128, 2, 4, N], mmdt, tag="w")
                nc.sync.dma_start(out=wp[:], in_=wt_in[P])

                if P + PPC < NPAIR:
                    bn1(P + PPC)
                if p == PPC - 1 and k + 1 < NCHUNK:
                    ln1_batch(k + 1)

                z_nxt = stage1_pre(P + 1) if P + 1 < NPAIR else None
                if dbg and P == 0:
                    nc.scalar.dma_start(out=z_dbg[:], in_=z_cur[:])
                tok = mix_pair(P, z_cur, wp,
                               b1p if not skip_b1 else None, skip_b1)
                stage1_post(P, tok)
                z_cur = z_nxt

                if p == PPC - 1:
                    ln2_batch(k)
                    for pj in range(PPC):
                        Pg = k * PPC + pj
                        nc.scalar.activation(
                            out=yn_pk[k][:, :, :, pj, :],
                            in_=u_pk[k][:, :, :, pj, :], func=IDENT,
                            bias=nmr2[:, Pg:Pg + 1],
                            scale=rstd2[:, Pg:Pg + 1])
                    g = grp_of[k]
                    kl = k - CCG[g][0]
                    for t, src in ((0, yn_pk[k]), (1, u_pk[k])):
                        for par in range(2):
                            for pr2 in range(2):
                                nc.sync.dma_start(
                                    out=sendb[g][:, t, pr2, :, kl,
                                                 par * PPC:
                                                 (par + 1) * PPC,
                                                 :].rearrange(
                                                  "d b c n -> b d (c n)"),
                                    in_=src[par * 64:(par + 1) * 64, :,
                                            pr2].rearrange(
                                                "q d c n -> q d (c n)"))
                    if k == CCG[g][-1]:
                        nc.gpsimd.collective_compute(
                            "AllToAll", mybir.AluOpType.bypass,
                            replica_groups=[list(range(NCORE))],
                            ins=[sendb[g].opt()], outs=[recvb[g].opt()])

            if dbg:
                nc.scalar.dma_start(out=rstd_dbg[:, 0:NPAIR], in_=rstd1[:])
                nc.scalar.dma_start(out=rstd_dbg[:, NPAIR:], in_=rstd2[:])
                nc.scalar.dma_start(out=u_dbg[:], in_=u_pk[0][:])

            # ---------------- phase 2 staging ----------------
            # [128(par2=nl//16, b), c-global 256, nlh 16]
            yn2 = const.tile([128, C, 16], BF16)
            u2 = const.tile([128, C, 16], BF16)
            for g, ks in enumerate(CCG):
                for t, dst in ((0, yn2), (1, u2)):
                    for par2 in range(2):
                        # dst c positions d*32 + (ks[0]+kk)*8 + ci, nlh 16
                        dview = dst[par2 * 64:(par2 + 1) * 64].rearrange(
                            "q (d kk ci) n -> q d kk (ci n)",
                            d=NCORE, kk=NCHUNK)[:, :, ks[0]:ks[0] + len(ks)]
                        nc.sync.dma_start(
                            out=dview,
                            in_=recvb[g][:, t, par2].rearrange(
                                "d b kk c n -> b d kk (c n)"))
            y_pk = [const.tile([128, PPC, C], BF16, name=f"y{k}")
                    for k in range(NCHUNK)]
            if dbg:
                nc.scalar.dma_start(out=yn2_dbg[:], in_=yn2[:])
                nc.scalar.dma_start(out=u2_dbg[:], in_=u2[:])

            def stage2_pre(Q):
                z2p = ps.tile([128, 2, 128], F32, tag="zxp")
                for cb in range(2):
                    nc.tensor.matmul(
                        z2p[:, cb, :],
                        yn2[:, cb * 128:(cb + 1) * 128, Q],
                        id128[:], start=True, stop=True)
                z2 = act.tile([128, 2, 128], BF16, tag="z")
                nc.vector.tensor_copy(out=z2[:], in_=z2p[:])
                return z2

            z2_cur = stage2_pre(0)
            for Q in range(NPAIR):
                k, q = divmod(Q, PPC)
                cp = wpool.tile([128, 2, 4, C], mmdt, tag="w")
                nc.scalar.dma_start(out=cp[:], in_=ct_in[Q])

                z2_nxt = stage2_pre(Q + 1) if Q + 1 < NPAIR else None
                ch_ps = mix_pair(Q, z2_cur, cp,
                                 bc1p if not skip_bc1 else None, skip_bc1)
                nc.vector.scalar_tensor_tensor(
                    out=y_pk[k][:, q, :], in0=ch_ps, scalar=wsi,
                    in1=u2[:, :, Q], op0=MUL, op1=ADD)
                z2_cur = z2_nxt
                if q == PPC - 1:
                    for par2 in range(2):
                        nc.scalar.dma_start(
                            out=ybuf[:, par2 * 16 + k * PPC:
                                     par2 * 16 + (k + 1) * PPC, :],
                            in_=y_pk[k][par2 * 64:(par2 + 1) * 64])

    nc.finalize()
    return nc


def prep_inputs(x, g1, be1, g2, be2, tw1, tb1, tw2, tb2, cw1, cb1, cw2, cb2,
                mmdt_np=ml_dtypes.bfloat16, ws=1.0):
    """Host-side sharding + weight folding. Returns in_maps for the 8 cores."""
    f = np.float32
    x = np.asarray(x, f)
    g1, be1, g2, be2 = (np.asarray(a, f) for a in (g1, be1, g2, be2))
    tw1, tb1, tw2, tb2 = (np.asarray(a, f) for a in (tw1, tb1, tw2, tb2))
    cw1, cb1, cw2, cb2 = (np.asarray(a, f) for a in (cw1, cb1, cw2, cb2))

    def wcast(a):
        a = a * ws
        if mmdt_np is not ml_dtypes.bfloat16:
            a = np.clip(a, -240.0, 240.0)
        return a.astype(mmdt_np)

    w1t = (tw1 * g1[None, None, :]).transpose(0, 2, 1)            # [C, N, M]
    bias1 = (tb1 + np.einsum('n,cmn->cm', be1, tw1)) * ws         # [C, M]
    w2t = tw2.transpose(0, 2, 1)                                  # [c, m, k]
    t1r = w1t.reshape(C, 2, 128, N)
    t2r = w2t.reshape(C, 2, 128, N)
    wt = np.ascontiguousarray(
        np.stack([t1r[:, 0], t1r[:, 1], t2r[:, 0], t2r[:, 1]],
                 axis=2))                                         # [C,128,4,N]

    c1t = (cw1 * g2[:, None, None]).transpose(0, 2, 1)            # [N, C, O]
    biasc1 = (cb1 + be2[:, None] * cw1.sum(axis=2)) * ws          # [N, O]
    c2t = cw2.transpose(0, 2, 1)                                  # [n, o, k]
    c1r = c1t.reshape(N, 2, 128, C)
    c2r = c2t.reshape(N, 2, 128, C)
    ct = np.ascontiguousarray(
        np.stack([c1r[:, 0], c1r[:, 1], c2r[:, 0], c2r[:, 1]],
                 axis=2))                                         # [N,128,4,C]

    id128 = np.eye(128, dtype=f).astype(ml_dtypes.bfloat16)
    msel = np.zeros((4, 2, 2, 64), f)
    for mb in range(2):
        for par in range(2):
            msel[mb * 2 + par, mb, par, :] = 1.0
    msel = msel.reshape(4, 256).astype(ml_dtypes.bfloat16)

    # channel pair order within a core: chunk k has channels k*8+ci,
    # pairs are (ci, ci+4); patch pairs are (nl, nl+16)
    cpair0 = np.array([k * CH + p for k in range(NCHUNK)
                       for p in range(PPC)])                      # 16
    npair0 = np.arange(16)

    def pair_pack(wfull, p0, off):   # [G,128,4,X] -> [G/2,128,2,4,X]
        a = wfull[p0]
        b = wfull[p0 + off]
        return np.ascontiguousarray(np.stack([a, b], axis=2))

    def bias_pair(bm, p0, off):      # [G,256] -> [4, G/2, 128] (mb*2+par)
        out = np.empty((4, len(p0), 128), f)
        for mb in range(2):
            for par in range(2):
                out[mb * 2 + par] = bm[p0 + par * off,
                                       mb * 128:(mb + 1) * 128]
        return np.ascontiguousarray(out).astype(ml_dtypes.bfloat16)

    in_maps = []
    for m in range(NCORE):
        cs = slice(m * CL, (m + 1) * CL)
        ns = slice(m * NL, (m + 1) * NL)
        xl = x[:, cs, :]                                          # [B,CL,N]
        # pair-packed x: [2(par), B, 16(chunk-major pairs), N]
        xp = np.stack([xl[:, cpair0, :], xl[:, cpair0 + 4, :]], axis=0)
        wtl = wcast(wt[cs])
        ctl = wcast(ct[ns])
        d = {
            "x_sh": np.ascontiguousarray(xp),
            "wt": pair_pack(wtl, cpair0, 4),
            "ct": pair_pack(ctl, npair0, 16),
            "id128": id128,
        }
        if np.any(bias1):
            d["b1p"] = bias_pair(bias1[cs], cpair0, 4)
            d["msel"] = msel
        if np.any(biasc1):
            d["bc1p"] = bias_pair(biasc1[ns], npair0, 16)
            d["msel"] = msel
        if np.any(tb2):
            d["b2r"] = (tb2[cs] * ws).astype(ml_dtypes.bfloat16)
        if np.any(cb2):
            d["bc2r"] = (cb2.T[ns] * ws).astype(ml_dtypes.bfloat16)
        in_maps.append(d)
    return in_maps


def assemble_output(results):
    """results: per-core dicts with 'ybuf' [B, NL, C] -> y [B, C, N]."""
    y = np.empty((B, C, N), np.float32)
    for k in range(NCORE):
        y[:, :, k * NL:(k + 1) * NL] = (
            results[k]["ybuf"].astype(np.float32).transpose(0, 2, 1))
    return y


_PROGRAMS = {}

# weight dtype config: (mybir dtype, numpy dtype, weight scale)
USE_FP8 = False
_W_CFG = ((mybir.dt.float8e4, ml_dtypes.float8_e4m3, 64.0) if USE_FP8
          else (BF16, ml_dtypes.bfloat16, 1.0))


def get_program(skip_b2, skip_bc2, skip_b1=True, skip_bc1=True):
    key = (skip_b1, skip_b2, skip_bc1, skip_bc2, USE_FP8)
    if key not in _PROGRAMS:
        _PROGRAMS[key] = build_program(
            mmdt=_W_CFG[0], ws=_W_CFG[2],
            skip_b1=skip_b1, skip_b2=skip_b2,
            skip_bc1=skip_bc1, skip_bc2=skip_bc2)
    return _PROGRAMS[key]


def kernel(**inputs):
    skip_b1 = not (np.any(np.asarray(inputs["tb1"]))
                   or np.any(np.asarray(inputs["be1"])))
    skip_bc1 = not (np.any(np.asarray(inputs["cb1"]))
                    or np.any(np.asarray(inputs["be2"])))
    skip_b2 = not np.any(np.asarray(inputs["tb2"]))
    skip_bc2 = not np.any(np.asarray(inputs["cb2"]))
    prog = get_program(skip_b2, skip_bc2, skip_b1, skip_bc1)
    in_maps = prep_inputs(**inputs, mmdt_np=_W_CFG[1], ws=_W_CFG[2])
    res = run_bass_kernel_spmd(prog, in_maps, list(range(NCORE)))
    return assemble_output(res.results)


if __name__ == "__main__":
    from scipy.special import erf

    rng = np.random.RandomState(0)
    s = 0.02
    inputs = dict(
        x=rng.randn(B, C, N).astype(np.float32),
        g1=np.ones(N, np.float32), be1=np.zeros(N, np.float32),
        g2=np.ones(N, np.float32), be2=np.zeros(N, np.float32),
        tw1=(rng.randn(C, N, N) * s).astype(np.float32),
        tb1=np.zeros((C, N), np.float32),
        tw2=(rng.randn(C, N, N) * s).astype(np.float32),
        tb2=np.zeros((C, N), np.float32),
        cw1=(rng.randn(N, C, C) * s).astype(np.float32),
        cb1=np.zeros((N, C), np.float32),
        cw2=(rng.randn(N, C, C) * s).astype(np.float32),
        cb2=np.zeros((N, C), np.float32),
    )

    def np_ref(x, g1, be1, g2, be2, tw1, tb1, tw2, tb2, cw1, cb1, cw2, cb2):
        def ln(z, g, b):
            mu = z.mean(-1, keepdims=True)
            var = z.var(-1, keepdims=True)
            return (z - mu) / np.sqrt(var + EPS) * g + b
        def gelu(v):
            return v * 0.5 * (1 + erf(v / np.sqrt(2.0)))
        xn = ln(x, g1, be1)
        h = gelu(np.einsum('bcn,cmn->bcm', xn, tw1) + tb1[None])
        tok = np.einsum('bcm,ckm->bck', h, tw2) + tb2[None]
        x = x + tok
        yn = ln(x, g2, be2)
        h2 = gelu(np.einsum('bcn,noc->bon', yn, cw1) + cb1.T[None])
        ch = np.einsum('bon,nko->bkn', h2, cw2) + cb2.T[None]
        return x + ch

    exp = np_ref(**{k: v.astype(np.float64) for k, v in inputs.items()})
    got = kernel(**inputs)
    err = np.abs(got - exp)
    rel = err.max() / np.abs(exp).max()
    print(f"abs err: {err.max():.3e}  rel(absmax): {rel:.3e}")


# revision 42
# speedup vs baseline: 1.1049x; 1.1049x over previous
"""Mixer (token-mix + channel-mix MLP) kernel for 8 TRN2 NeuronCores.

v3: pair-packed [batch, feature] design (expert-parallel over group axes).
  All elementwise/activation work runs on the full 128 partitions by packing
  two groups per op: phase 1 packs channels (ci, ci+4) of each 8-channel
  chunk as partitions (b, b+64); phase 2 packs patches (nl, nl+16).
  Per pair: LN1 via bn_stats/bn_aggr (one per iteration, pipelined),
  xn = x*rstd+nmr as one DVE tensor_scalar; ONE matmul per 128-block
  transposes both packed groups against a shared identity; fc1 is
  weight-stationary (bias-free when biases are zero); gelu is a single
  full-width activation; fc2 is weight-moving (256-wide moving operand)
  with the pair's outputs landing in one PSUM tile via tile_position
  column offsets; u = x + tok is one DVE scalar_tensor_tensor with fused
  sum; sum(u^2) one scalar Square with accum; yn = u*rstd2+nmr2 one
  activation. u and yn ship bf16 over 4 chunked AllToAlls that overlap the
  phase-1 tail. Phase 2 needs no normalization; y = u + ch per pair, output
  chunked. Weights stream as per-pair DMAs ([128, 2, 4, 256] blocks).
"""
import sys
import numpy as np

sys.path.insert(0, "/opt/trn_rl_repo")

import ml_dtypes
import concourse.bass as bass
import concourse.bacc as bacc
import concourse.tile as tile
from concourse import mybir
from concourse.bass_utils import run_bass_kernel_spmd

F32 = mybir.dt.float32
BF16 = mybir.dt.bfloat16
NCORE = 8
B, C, N = 64, 256, 256
CL = C // NCORE   # 32 local channels (phase 1)
NL = N // NCORE   # 32 local patches (phase 2)
EPS = 1e-5
GELU = mybir.ActivationFunctionType.Gelu
IDENT = mybir.ActivationFunctionType.Identity
SQRT = mybir.ActivationFunctionType.Sqrt
SQUARE = mybir.ActivationFunctionType.Square
MUL = mybir.AluOpType.mult
ADD = mybir.AluOpType.add

CH = 8                 # channels per stats/collective chunk
NCHUNK = CL // CH      # 4 chunks
PPC = 4                # pairs per chunk
NPAIR = CL // 2        # 16 channel pairs / 16 patch pairs
BLK = CH * NL          # 256 elems per (dest, tensor) block row


def build_program(gelu_func=GELU, mmdt=BF16, ws=1.0, skip_b1=True,
                  skip_b2=True, skip_bc1=True, skip_bc2=True, dbg=False):
    nc = bacc.Bacc("TRN2", target_bir_lowering=False, debug=False,
                   enable_asserts=True, num_devices=NCORE)
    wsi = 1.0 / ws

    x_in = nc.dram_tensor("x_sh", [2, B, PPC * NCHUNK, N], F32,
                          kind="ExternalInput")
    wt_in = nc.dram_tensor("wt", [NPAIR, 128, 2, 4, N], mmdt,
                           kind="ExternalInput")
    ct_in = nc.dram_tensor("ct", [NPAIR, 128, 2, 4, C], mmdt,
                           kind="ExternalInput")
    id128_in = nc.dram_tensor("id128", [128, 128], BF16,
                              kind="ExternalInput")
    if not skip_b1:
        b1p_in = nc.dram_tensor("b1p", [4, NPAIR, 128], BF16,
                                kind="ExternalInput")
    if not (skip_b1 and skip_bc1):
        msel_in = nc.dram_tensor("msel", [4, 256], BF16,
                                 kind="ExternalInput")
    if not skip_bc1:
        bc1p_in = nc.dram_tensor("bc1p", [4, NPAIR, 128], BF16,
                                 kind="ExternalInput")
    if not skip_b2:
        b2r_in = nc.dram_tensor("b2r", [CL, N], BF16, kind="ExternalInput")
    if not skip_bc2:
        bc2r_in = nc.dram_tensor("bc2r", [NL, C], BF16, kind="ExternalInput")

    ybuf = nc.dram_tensor("ybuf", [B, NL, C], BF16, kind="ExternalOutput")
    if dbg:
        xn_dbg = nc.dram_tensor("xn_dbg", [128, N], BF16,
                                kind="ExternalOutput")
        z_dbg = nc.dram_tensor("z_dbg", [128, 2, 128], BF16,
                               kind="ExternalOutput")
        rstd_dbg = nc.dram_tensor("rstd_dbg", [128, 2 * NPAIR], F32,
                                  kind="ExternalOutput")
        u_dbg = nc.dram_tensor("u_dbg", [128, NCORE, 2, PPC, 16], BF16,
                               kind="ExternalOutput")
        yn2_dbg = nc.dram_tensor("yn2_dbg", [128, C, 16], BF16,
                                 kind="ExternalOutput")
        u2_dbg = nc.dram_tensor("u2_dbg", [128, C, 16], BF16,
                                kind="ExternalOutput")

    with tile.TileContext(nc) as tc:
        with tc.tile_pool(name="const", bufs=1) as const, \
             tc.tile_pool(name="wpool", bufs=6) as wpool, \
             tc.tile_pool(name="act", bufs=3) as act, \
             tc.tile_pool(name="scr", bufs=2) as scr, \
             tc.tile_pool(name="dram", bufs=1, space="DRAM") as dram, \
             tc.tile_pool(name="ps", bufs=2, space="PSUM") as ps:

            # block: [dest, {yn,u}, nl-half, b, chunk-in-half, ci, nlh]
            # two collectives: half h carries chunks {2h, 2h+1}
            sendb = [dram.tile([NCORE, 2, 2, B, 2, CH, 16], BF16,
                               name=f"snd{h}") for h in range(2)]
            recvb = [dram.tile([NCORE, 2, 2, B, 2, CH, 16], BF16,
                               name=f"rcv{h}") for h in range(2)]

            id128 = const.tile([128, 128], BF16)
            nc.sync.dma_start(out=id128[:], in_=id128_in[:])
            if not skip_b1:
                b1p = const.tile([4, NPAIR, 128], BF16)
                nc.sync.dma_start(out=b1p[:], in_=b1p_in[:])
            if not (skip_b1 and skip_bc1):
                msel = const.tile([4, 256], BF16)
                nc.sync.dma_start(out=msel[:], in_=msel_in[:])
            if not skip_bc1:
                bc1p = const.tile([4, NPAIR, 128], BF16)
                nc.sync.dma_start(out=bc1p[:], in_=bc1p_in[:])
            if not skip_b2:
                b2r = const.tile([CL, N], BF16)
                nc.sync.dma_start(out=b2r[:], in_=b2r_in[:])
                ones1 = const.tile([1, 64], BF16)
                nc.vector.memset(ones1[:], 1.0)
            if not skip_bc2:
                bc2r = const.tile([NL, C], BF16)
                nc.sync.dma_start(out=bc2r[:], in_=bc2r_in[:])
                ones1c = const.tile([1, 64], BF16)
                nc.vector.memset(ones1c[:], 1.0)
            eps128 = const.tile([128, 1], F32)
            nc.vector.memset(eps128[:], EPS)

            # x pair-packed: partition (par, b); par=ci//4 within chunk
            x_sb = [const.tile([128, PPC, N], F32, name=f"x{k}")
                    for k in range(NCHUNK)]
            for k in range(NCHUNK):
                for par in range(2):
                    nc.sync.dma_start(
                        out=x_sb[k][par * 64:(par + 1) * 64, :, :],
                        in_=x_in[par, :, k * PPC:(k + 1) * PPC, :])

            # [128(par,b), dest, nl-half, pair, nlh]
            u_pk = [const.tile([128, NCORE, 2, PPC, 16], BF16,
                               name=f"u{k}") for k in range(NCHUNK)]
            yn_pk = [const.tile([128, NCORE, 2, PPC, 16], BF16,
                                name=f"yn{k}") for k in range(NCHUNK)]

            # LN stats, one column per pair P
            st2 = const.tile([128, NPAIR, 2], F32)   # bn_aggr (mean, var)
            rstd1 = const.tile([128, NPAIR], F32)
            nmr1 = const.tile([128, NPAIR], F32)
            s2 = const.tile([128, NPAIR], F32)
            s2q = const.tile([128, NPAIR], F32)
            mu2 = const.tile([128, NPAIR], F32)
            rstd2 = const.tile([128, NPAIR], F32)
            nmr2 = const.tile([128, NPAIR], F32)
            tv = const.tile([128, NPAIR], F32)
            ts_ = const.tile([128, NPAIR], F32)

            def bn1(P):
                k, p = divmod(P, PPC)
                st6 = scr.tile([128, 6], F32, tag="st6")
                nc.vector.bn_stats(out=st6[:], in_=x_sb[k][:, p, :])
                nc.vector.bn_aggr(out=st2[:, P, :], in_=st6[:])

            def ln1_batch(k):
                cs = slice(k * PPC, (k + 1) * PPC)
                nc.scalar.activation(out=ts_[:, cs], in_=st2[:, cs, 1],
                                     func=SQRT, bias=eps128[:], scale=1.0)
                nc.vector.reciprocal(out=rstd1[:, cs], in_=ts_[:, cs])
                nc.vector.scalar_tensor_tensor(
                    out=nmr1[:, cs], in0=st2[:, cs, 0], scalar=-1.0,
                    in1=rstd1[:, cs], op0=MUL, op1=MUL)

            def ln2_batch(k):
                cs = slice(k * PPC, (k + 1) * PPC)
                nc.vector.tensor_scalar_mul(out=mu2[:, cs], in0=s2[:, cs],
                                            scalar1=1.0 / N)
                nc.vector.tensor_scalar_mul(out=tv[:, cs], in0=s2q[:, cs],
                                            scalar1=1.0 / N)
                nc.vector.tensor_mul(out=ts_[:, cs], in0=mu2[:, cs],
                                     in1=mu2[:, cs])
                nc.vector.tensor_sub(out=tv[:, cs], in0=tv[:, cs],
                                     in1=ts_[:, cs])
                nc.scalar.activation(out=ts_[:, cs], in_=tv[:, cs],
                                     func=SQRT, bias=eps128[:], scale=1.0)
                nc.vector.reciprocal(out=rstd2[:, cs], in_=ts_[:, cs])
                nc.vector.scalar_tensor_tensor(
                    out=nmr2[:, cs], in0=mu2[:, cs], scalar=-1.0,
                    in1=rstd2[:, cs], op0=MUL, op1=MUL)

            def stage1_pre(P):
                """xn + pair transpose + z copy for pair P."""
                k, p = divmod(P, PPC)
                xn = act.tile([128, N], BF16, tag="xn")
                nc.vector.tensor_scalar(
                    out=xn[:], in0=x_sb[k][:, p, :],
                    scalar1=rstd1[:, P:P + 1], scalar2=nmr1[:, P:P + 1],
                    op0=MUL, op1=ADD)
                zxp = ps.tile([128, 2, 128], F32, tag="zxp")
                for blk in range(2):
                    nc.tensor.matmul(
                        zxp[:, blk, :],
                        xn[:, blk * 128:(blk + 1) * 128],
                        id128[:], start=True, stop=True)
                z_sb = act.tile([128, 2, 128], BF16, tag="z")
                nc.vector.tensor_copy(out=z_sb[:], in_=zxp[:])
                if dbg and P == 0:
                    nc.scalar.dma_start(out=xn_dbg[:], in_=xn[:])
                return z_sb

            def mix_pair(P, z_sb, wp, bp, skip_b):
                """fc1 + gelu + fc2 for both groups of pair P."""
                hpre = ps.tile([128, 2, 2, 64], F32, tag="hpre")
                if not skip_b:
                    nc.tensor.matmul(
                        hpre[:].rearrange("p a b c -> p (a b c)"),
                        bp[:, P, :], msel[:],
                        start=True, stop=False, skip_group_check=True)
                for par in range(2):
                    for mb in range(2):
                        for nb in range(2):
                            nc.tensor.matmul(
                                hpre[:, mb, par, :],
                                wp[:, par, nb, mb * 128:(mb + 1) * 128],
                                z_sb[:, nb, par * 64:(par + 1) * 64],
                                start=(skip_b and nb == 0), stop=(nb == 1),
                                skip_group_check=True)
                hs = act.tile([128, 2, 2, 64], BF16, tag="h")
                nc.scalar.activation(out=hs[:], in_=hpre[:], func=gelu_func,
                                     scale=wsi)

                tok = ps.tile([128, 256], F32, tag="tok")
                for par in range(2):
                    for mb in range(2):
                        nc.tensor.matmul(
                            tok[par * 64:(par + 1) * 64, :],
                            hs[:, mb, par, :],
                            wp[:, par, 2 + mb, :],
                            start=(mb == 0), stop=(mb == 1),
                            skip_group_check=True)
                return tok

            def stage1_post(P, tok):
                k, p = divmod(P, PPC)
                tok3 = tok.rearrange("q (d a n) -> q d a n", d=NCORE, a=2)
                x3 = x_sb[k][:, p, :].rearrange("q (d a n) -> q d a n",
                                                d=NCORE, a=2)
                nc.vector.scalar_tensor_tensor(
                    out=u_pk[k][:, :, :, p, :], in0=tok3, scalar=wsi,
                    in1=x3, op0=MUL, op1=ADD, accum_out=s2[:, P:P + 1])
                sqs = scr.tile([128, NCORE, 2, 16], BF16, tag="sqs")
                nc.scalar.activation(out=sqs[:], in_=u_pk[k][:, :, :, p, :],
                                     func=SQUARE,
                                     accum_out=s2q[:, P:P + 1])

            # ---------------- phase 1 ----------------
            for P in range(PPC):
                bn1(P)
            ln1_batch(0)
            z_cur = stage1_pre(0)
            for P in range(NPAIR):
                k, p = divmod(P, PPC)
                wp = wpool.tile([128, 2, 4, N], mmdt, tag="w")
                nc.sync.dma_start(out=wp[:], in_=wt_in[P])

                if P + PPC < NPAIR:
                    bn1(P + PPC)
                if p == PPC - 1 and k + 1 < NCHUNK:
                    ln1_batch(k + 1)

                z_nxt = stage1_pre(P + 1) if P + 1 < NPAIR else None
                if dbg and P == 0:
                    nc.scalar.dma_start(out=z_dbg[:], in_=z_cur[:])
                tok = mix_pair(P, z_cur, wp,
                               b1p if not skip_b1 else None, skip_b1)
                stage1_post(P, tok)
                z_cur = z_nxt

                if p == PPC - 1:
                    ln2_batch(k)
                    for pj in range(PPC):
                        Pg = k * PPC + pj
                        nc.scalar.activation(
                            out=yn_pk[k][:, :, :, pj, :],
                            in_=u_pk[k][:, :, :, pj, :], func=IDENT,
                            bias=nmr2[:, Pg:Pg + 1],
                            scale=rstd2[:, Pg:Pg + 1])
                    for t, src in ((0, yn_pk[k]), (1, u_pk[k])):
                        for par in range(2):
                            for pr2 in range(2):
                                nc.sync.dma_start(
                                    out=sendb[k // 2][:, t, pr2, :, k % 2,
                                                      par * PPC:
                                                      (par + 1) * PPC,
                                                      :].rearrange(
                                                  "d b c n -> b d (c n)"),
                                    in_=src[par * 64:(par + 1) * 64, :,
                                            pr2].rearrange(
                                                "q d c n -> q d (c n)"))
                    if k % 2 == 1:
                        h = k // 2
                        nc.gpsimd.collective_compute(
                            "AllToAll", mybir.AluOpType.bypass,
                            replica_groups=[list(range(NCORE))],
                            ins=[sendb[h].opt()], outs=[recvb[h].opt()])

            if dbg:
                nc.scalar.dma_start(out=rstd_dbg[:, 0:NPAIR], in_=rstd1[:])
                nc.scalar.dma_start(out=rstd_dbg[:, NPAIR:], in_=rstd2[:])
                nc.scalar.dma_start(out=u_dbg[:], in_=u_pk[0][:])

            # ---------------- phase 2 staging ----------------
            # [128(par2=nl//16, b), c-global 256, nlh 16]
            yn2 = const.tile([128, C, 16], BF16)
            u2 = const.tile([128, C, 16], BF16)
            for h in range(2):
                for t, dst in ((0, yn2), (1, u2)):
                    for par2 in range(2):
                        # dst c positions d*32 + (2h+kk)*8 + ci, nlh 16
                        dview = dst[par2 * 64:(par2 + 1) * 64].rearrange(
                            "q (d hh kk ci) n -> q hh d (kk ci n)",
                            d=NCORE, hh=2, kk=2)[:, h]
                        nc.sync.dma_start(
                            out=dview,
                            in_=recvb[h][:, t, par2].rearrange(
                                "d b kk c n -> b d (kk c n)"))
            y_pk = [const.tile([128, PPC, C], BF16, name=f"y{k}")
                    for k in range(NCHUNK)]
            if dbg:
                nc.scalar.dma_start(out=yn2_dbg[:], in_=yn2[:])
                nc.scalar.dma_start(out=u2_dbg[:], in_=u2[:])

            def stage2_pre(Q):
                z2p = ps.tile([128, 2, 128], F32, tag="zxp")
                for cb in range(2):
                    nc.tensor.matmul(
                        z2p[:, cb, :],
                        yn2[:, cb * 128:(cb + 1) * 128, Q],
                        id128[:], start=True, stop=True)
                z2 = act.tile([128, 2, 128], BF16, tag="z")
                nc.vector.tensor_copy(out=z2[:], in_=z2p[:])
                return z2

            z2_cur = stage2_pre(0)
            for Q in range(NPAIR):
                k, q = divmod(Q, PPC)
                cp = wpool.tile([128, 2, 4, C], mmdt, tag="w")
                nc.scalar.dma_start(out=cp[:], in_=ct_in[Q])

                z2_nxt = stage2_pre(Q + 1) if Q + 1 < NPAIR else None
                ch_ps = mix_pair(Q, z2_cur, cp,
                                 bc1p if not skip_bc1 else None, skip_bc1)
                nc.vector.scalar_tensor_tensor(
                    out=y_pk[k][:, q, :], in0=ch_ps, scalar=wsi,
                    in1=u2[:, :, Q], op0=MUL, op1=ADD)
                z2_cur = z2_nxt
                if q == PPC - 1:
                    for par2 in range(2):
                        nc.scalar.dma_start(
                            out=ybuf[:, par2 * 16 + k * PPC:
                                     par2 * 16 + (k + 1) * PPC, :],
                            in_=y_pk[k][par2 * 64:(par2 + 1) * 64])

    nc.finalize()
    return nc


def prep_inputs(x, g1, be1, g2, be2, tw1, tb1, tw2, tb2, cw1, cb1, cw2, cb2,
                mmdt_np=ml_dtypes.bfloat16, ws=1.0):
    """Host-side sharding + weight folding. Returns in_maps for the 8 cores."""
    f = np.float32
    x = np.asarray(x, f)
    g1, be1, g2, be2 = (np.asarray(a, f) for a in (g1, be1, g2, be2))
    tw1, tb1, tw2, tb2 = (np.asarray(a, f) for a in (tw1, tb1, tw2, tb2))
    cw1, cb1, cw2, cb2 = (np.asarray(a, f) for a in (cw1, cb1, cw2, cb2))

    def wcast(a):
        a = a * ws
        if mmdt_np is not ml_dtypes.bfloat16:
            a = np.clip(a, -240.0, 240.0)
        return a.astype(mmdt_np)

    w1t = (tw1 * g1[None, None, :]).transpose(0, 2, 1)            # [C, N, M]
    bias1 = (tb1 + np.einsum('n,cmn->cm', be1, tw1)) * ws         # [C, M]
    w2t = tw2.transpose(0, 2, 1)                                  # [c, m, k]
    t1r = w1t.reshape(C, 2, 128, N)
    t2r = w2t.reshape(C, 2, 128, N)
    wt = np.ascontiguousarray(
        np.stack([t1r[:, 0], t1r[:, 1], t2r[:, 0], t2r[:, 1]],
                 axis=2))                                         # [C,128,4,N]

    c1t = (cw1 * g2[:, None, None]).transpose(0, 2, 1)            # [N, C, O]
    biasc1 = (cb1 + be2[:, None] * cw1.sum(axis=2)) * ws          # [N, O]
    c2t = cw2.transpose(0, 2, 1)                                  # [n, o, k]
    c1r = c1t.reshape(N, 2, 128, C)
    c2r = c2t.reshape(N, 2, 128, C)
    ct = np.ascontiguousarray(
        np.stack([c1r[:, 0], c1r[:, 1], c2r[:, 0], c2r[:, 1]],
                 axis=2))                                         # [N,128,4,C]

    id128 = np.eye(128, dtype=f).astype(ml_dtypes.bfloat16)
    msel = np.zeros((4, 2, 2, 64), f)
    for mb in range(2):
        for par in range(2):
            msel[mb * 2 + par, mb, par, :] = 1.0
    msel = msel.reshape(4, 256).astype(ml_dtypes.bfloat16)

    # channel pair order within a core: chunk k has channels k*8+ci,
    # pairs are (ci, ci+4); patch pairs are (nl, nl+16)
    cpair0 = np.array([k * CH + p for k in range(NCHUNK)
                       for p in range(PPC)])                      # 16
    npair0 = np.arange(16)

    def pair_pack(wfull, p0, off):   # [G,128,4,X] -> [G/2,128,2,4,X]
        a = wfull[p0]
        b = wfull[p0 + off]
        return np.ascontiguousarray(np.stack([a, b], axis=2))

    def bias_pair(bm, p0, off):      # [G,256] -> [4, G/2, 128] (mb*2+par)
        out = np.empty((4, len(p0), 128), f)
        for mb in range(2):
            for par in range(2):
                out[mb * 2 + par] = bm[p0 + par * off,
                                       mb * 128:(mb + 1) * 128]
        return np.ascontiguousarray(out).astype(ml_dtypes.bfloat16)

    in_maps = []
    for m in range(NCORE):
        cs = slice(m * CL, (m + 1) * CL)
        ns = slice(m * NL, (m + 1) * NL)
        xl = x[:, cs, :]                                          # [B,CL,N]
        # pair-packed x: [2(par), B, 16(chunk-major pairs), N]
        xp = np.stack([xl[:, cpair0, :], xl[:, cpair0 + 4, :]], axis=0)
        wtl = wcast(wt[cs])
        ctl = wcast(ct[ns])
        d = {
            "x_sh": np.ascontiguousarray(xp),
            "wt": pair_pack(wtl, cpair0, 4),
            "ct": pair_pack(ctl, npair0, 16),
            "id128": id128,
        }
        if np.any(bias1):
            d["b1p"] = bias_pair(bias1[cs], cpair0, 4)
            d["msel"] = msel
        if np.any(biasc1):
            d["bc1p"] = bias_pair(biasc1[ns], npair0, 16)
            d["msel"] = msel
        if np.any(tb2):
            d["b2r"] = (tb2[cs] * ws).astype(ml_dtypes.bfloat16)
        if np.any(cb2):
            d["bc2r"] = (cb2.T[ns] * ws).astype(ml_dtypes.bfloat16)
        in_maps.append(d)
    return in_maps


def assemble_output(results):
    """results: per-core dicts with 'ybuf' [B, NL, C] -> y [B, C, N]."""
    y = np.empty((B, C, N), np.float32)
    for k in range(NCORE):
        y[:, :, k * NL:(k + 1) * NL] = (
            results[k]["ybuf"].astype(np.float32).transpose(0, 2, 1))
    return y


_PROGRAMS = {}

# weight dtype config: (mybir dtype, numpy dtype, weight scale)
USE_FP8 = False
_W_CFG = ((mybir.dt.float8e4, ml_dtypes.float8_e4m3, 64.0) if USE_FP8
          else (BF16, ml_dtypes.bfloat16, 1.0))


def get_program(skip_b2, skip_bc2, skip_b1=True, skip_bc1=True):
    key = (skip_b1, skip_b2, skip_bc1, skip_bc2, USE_FP8)
    if key not in _PROGRAMS:
        _PROGRAMS[key] = build_program(
            mmdt=_W_CFG[0], ws=_W_CFG[2],
            skip_b1=skip_b1, skip_b2=skip_b2,
            skip_bc1=skip_bc1, skip_bc2=skip_bc2)
    return _PROGRAMS[key]


def kernel(**inputs):
    skip_b1 = not (np.any(np.asarray(inputs["tb1"]))
                   or np.any(np.asarray(inputs["be1"])))
    skip_bc1 = not (np.any(np.asarray(inputs["cb1"]))
                    or np.any(np.asarray(inputs["be2"])))
    skip_b2 = not np.any(np.asarray(inputs["tb2"]))
    skip_bc2 = not np.any(np.asarray(inputs["cb2"]))
    prog = get_program(skip_b2, skip_bc2, skip_b1, skip_bc1)
    in_maps = prep_inputs(**inputs, mmdt_np=_W_CFG[1], ws=_W_CFG[2])
    res = run_bass_kernel_spmd(prog, in_maps, list(range(NCORE)))
    return assemble_output(res.results)


if __name__ == "__main__":
    from scipy.special import erf

    rng = np.random.RandomState(0)
    s = 0.02
    inputs = dict(
        x=rng.randn(B, C, N).astype(np.float32),
        g1=np.ones(N, np.float32), be1=np.zeros(N, np.float32),
        g2=np.ones(N, np.float32), be2=np.zeros(N, np.float32),
        tw1=(rng.randn(C, N, N) * s).astype(np.float32),
        tb1=np.zeros((C, N), np.float32),
        tw2=(rng.randn(C, N, N) * s).astype(np.float32),
        tb2=np.zeros((C, N), np.float32),
        cw1=(rng.randn(N, C, C) * s).astype(np.float32),
        cb1=np.zeros((N, C), np.float32),
        cw2=(rng.randn(N, C, C) * s).astype(np.float32),
        cb2=np.zeros((N, C), np.float32),
    )

    def np_ref(x, g1, be1, g2, be2, tw1, tb1, tw2, tb2, cw1, cb1, cw2, cb2):
        def ln(z, g, b):
            mu = z.mean(-1, keepdims=True)
            var = z.var(-1, keepdims=True)
            return (z - mu) / np.sqrt(var + EPS) * g + b
        def gelu(v):
            return v * 0.5 * (1 + erf(v / np.sqrt(2.0)))
        xn = ln(x, g1, be1)
        h = gelu(np.einsum('bcn,cmn->bcm', xn, tw1) + tb1[None])
        tok = np.einsum('bcm,ckm->bck', h, tw2) + tb2[None]
        x = x + tok
        yn = ln(x, g2, be2)
        h2 = gelu(np.einsum('bcn,noc->bon', yn, cw1) + cb1.T[None])
        ch = np.einsum('bon,nko->bkn', h2, cw2) + cb2.T[None]
        return x + ch

    exp = np_ref(**{k: v.astype(np.float64) for k, v in inputs.items()})
    got = kernel(**inputs)
    err = np.abs(got - exp)
    rel = err.max() / np.abs(exp).max()
    print(f"abs err: {err.max():.3e}  rel(absmax): {rel:.3e}")


# revision 47
# speedup vs baseline: 1.1606x; 1.0503x over previous
"""Mixer (token-mix + channel-mix MLP) kernel for 8 TRN2 NeuronCores.

v3: pair-packed [batch, feature] design (expert-parallel over group axes).
  All elementwise/activation work runs on the full 128 partitions by packing
  two groups per op: phase 1 packs channels (ci, ci+4) of each 8-channel
  chunk as partitions (b, b+64); phase 2 packs patches (nl, nl+16).
  Per pair: LN1 via bn_stats/bn_aggr (one per iteration, pipelined),
  xn = x*rstd+nmr as one DVE tensor_scalar; ONE matmul per 128-block
  transposes both packed groups against a shared identity; fc1 is
  weight-stationary (bias-free when biases are zero); gelu is a single
  full-width activation; fc2 is weight-moving (256-wide moving operand)
  with the pair's outputs landing in one PSUM tile via tile_position
  column offsets; u = x + tok is one DVE scalar_tensor_tensor with fused
  sum; sum(u^2) one scalar Square with accum; yn = u*rstd2+nmr2 one
  activation. u and yn ship bf16 over 4 chunked AllToAlls that overlap the
  phase-1 tail. Phase 2 needs no normalization; y = u + ch per pair, output
  chunked. Weights stream as per-pair DMAs ([128, 2, 4, 256] blocks).
"""
import sys
import numpy as np

sys.path.insert(0, "/opt/trn_rl_repo")

import ml_dtypes
import concourse.bass as bass
import concourse.bacc as bacc
import concourse.tile as tile
from concourse import mybir
from concourse.bass_utils import run_bass_kernel_spmd

F32 = mybir.dt.float32
BF16 = mybir.dt.bfloat16
NCORE = 8
B, C, N = 64, 256, 256
CL = C // NCORE   # 32 local channels (phase 1)
NL = N // NCORE   # 32 local patches (phase 2)
EPS = 1e-5
GELU = mybir.ActivationFunctionType.Gelu
IDENT = mybir.ActivationFunctionType.Identity
SQRT = mybir.ActivationFunctionType.Sqrt
SQUARE = mybir.ActivationFunctionType.Square
MUL = mybir.AluOpType.mult
ADD = mybir.AluOpType.add

CH = 8                 # channels per stats/collective chunk
NCHUNK = CL // CH      # 4 chunks
PPC = 4                # pairs per chunk
NPAIR = CL // 2        # 16 channel pairs / 16 patch pairs
BLK = CH * NL          # 256 elems per (dest, tensor) block row


def build_program(gelu_func=GELU, mmdt=BF16, ws=1.0, skip_b1=True,
                  skip_b2=True, skip_bc1=True, skip_bc2=True, dbg=False):
    nc = bacc.Bacc("TRN2", target_bir_lowering=False, debug=False,
                   enable_asserts=True, num_devices=NCORE)
    wsi = 1.0 / ws

    x_in = nc.dram_tensor("x_sh", [2, B, PPC * NCHUNK, N], F32,
                          kind="ExternalInput")
    wt_in = nc.dram_tensor("wt", [NPAIR, 128, 2, 4, N], mmdt,
                           kind="ExternalInput")
    ct_in = nc.dram_tensor("ct", [NPAIR, 128, 2, 4, C], mmdt,
                           kind="ExternalInput")
    id128_in = nc.dram_tensor("id128", [128, 128], BF16,
                              kind="ExternalInput")
    if not skip_b1:
        b1p_in = nc.dram_tensor("b1p", [4, NPAIR, 128], BF16,
                                kind="ExternalInput")
    if not (skip_b1 and skip_bc1):
        msel_in = nc.dram_tensor("msel", [4, 256], BF16,
                                 kind="ExternalInput")
    if not skip_bc1:
        bc1p_in = nc.dram_tensor("bc1p", [4, NPAIR, 128], BF16,
                                 kind="ExternalInput")
    if not skip_b2:
        b2r_in = nc.dram_tensor("b2r", [CL, N], BF16, kind="ExternalInput")
    if not skip_bc2:
        bc2r_in = nc.dram_tensor("bc2r", [NL, C], BF16, kind="ExternalInput")

    ybuf = nc.dram_tensor("ybuf", [B, NL, C], BF16, kind="ExternalOutput")
    if dbg:
        xn_dbg = nc.dram_tensor("xn_dbg", [128, N], BF16,
                                kind="ExternalOutput")
        z_dbg = nc.dram_tensor("z_dbg", [128, 2, 128], BF16,
                               kind="ExternalOutput")
        rstd_dbg = nc.dram_tensor("rstd_dbg", [128, 2 * NPAIR], F32,
                                  kind="ExternalOutput")
        u_dbg = nc.dram_tensor("u_dbg", [128, NCORE, 2, PPC, 16], BF16,
                               kind="ExternalOutput")
        yn2_dbg = nc.dram_tensor("yn2_dbg", [128, C, 16], BF16,
                                 kind="ExternalOutput")
        u2_dbg = nc.dram_tensor("u2_dbg", [128, C, 16], BF16,
                                kind="ExternalOutput")

    with tile.TileContext(nc) as tc:
        with tc.tile_pool(name="const", bufs=1) as const, \
             tc.tile_pool(name="wpool", bufs=10) as wpool, \
             tc.tile_pool(name="act", bufs=4) as act, \
             tc.tile_pool(name="scr", bufs=2) as scr, \
             tc.tile_pool(name="dram", bufs=1, space="DRAM") as dram, \
             tc.tile_pool(name="ps", bufs=2, space="PSUM") as ps:

            # block: [dest, {yn,u}, nl-half, b, chunk-in-half, ci, nlh]
            # two collectives: half h carries chunks {2h, 2h+1}
            sendb = [dram.tile([NCORE, 2, 2, B, 2, CH, 16], BF16,
                               name=f"snd{h}") for h in range(2)]
            recvb = [dram.tile([NCORE, 2, 2, B, 2, CH, 16], BF16,
                               name=f"rcv{h}") for h in range(2)]

            id128 = const.tile([128, 128], BF16)
            nc.sync.dma_start(out=id128[:], in_=id128_in[:])
            if not skip_b1:
                b1p = const.tile([4, NPAIR, 128], BF16)
                nc.sync.dma_start(out=b1p[:], in_=b1p_in[:])
            if not (skip_b1 and skip_bc1):
                msel = const.tile([4, 256], BF16)
                nc.sync.dma_start(out=msel[:], in_=msel_in[:])
            if not skip_bc1:
                bc1p = const.tile([4, NPAIR, 128], BF16)
                nc.sync.dma_start(out=bc1p[:], in_=bc1p_in[:])
            if not skip_b2:
                b2r = const.tile([CL, N], BF16)
                nc.sync.dma_start(out=b2r[:], in_=b2r_in[:])
                ones1 = const.tile([1, 64], BF16)
                nc.vector.memset(ones1[:], 1.0)
            if not skip_bc2:
                bc2r = const.tile([NL, C], BF16)
                nc.sync.dma_start(out=bc2r[:], in_=bc2r_in[:])
                ones1c = const.tile([1, 64], BF16)
                nc.vector.memset(ones1c[:], 1.0)
            eps128 = const.tile([128, 1], F32)
            nc.vector.memset(eps128[:], EPS)

            # x pair-packed: partition (par, b); par=ci//4 within chunk
            x_sb = [const.tile([128, PPC, N], F32, name=f"x{k}")
                    for k in range(NCHUNK)]
            for k in range(NCHUNK):
                for par in range(2):
                    nc.sync.dma_start(
                        out=x_sb[k][par * 64:(par + 1) * 64, :, :],
                        in_=x_in[par, :, k * PPC:(k + 1) * PPC, :])

            # [128(par,b), dest, nl-half, pair, nlh]
            u_pk = [const.tile([128, NCORE, 2, PPC, 16], BF16,
                               name=f"u{k}") for k in range(NCHUNK)]
            yn_pk = [const.tile([128, NCORE, 2, PPC, 16], BF16,
                                name=f"yn{k}") for k in range(NCHUNK)]

            # LN stats, one column per pair P
            st2 = const.tile([128, NPAIR, 2], F32)   # bn_aggr (mean, var)
            rstd1 = const.tile([128, NPAIR], F32)
            nmr1 = const.tile([128, NPAIR], F32)
            s2 = const.tile([128, NPAIR], F32)
            s2q = const.tile([128, NPAIR], F32)
            mu2 = const.tile([128, NPAIR], F32)
            rstd2 = const.tile([128, NPAIR], F32)
            nmr2 = const.tile([128, NPAIR], F32)
            tv = const.tile([128, NPAIR], F32)
            ts_ = const.tile([128, NPAIR], F32)

            def bn1(P):
                k, p = divmod(P, PPC)
                st6 = scr.tile([128, 6], F32, tag="st6")
                nc.vector.bn_stats(out=st6[:], in_=x_sb[k][:, p, :])
                nc.vector.bn_aggr(out=st2[:, P, :], in_=st6[:])

            def ln1_batch(k):
                cs = slice(k * PPC, (k + 1) * PPC)
                nc.scalar.activation(out=ts_[:, cs], in_=st2[:, cs, 1],
                                     func=SQRT, bias=eps128[:], scale=1.0)
                nc.vector.reciprocal(out=rstd1[:, cs], in_=ts_[:, cs])
                nc.vector.scalar_tensor_tensor(
                    out=nmr1[:, cs], in0=st2[:, cs, 0], scalar=-1.0,
                    in1=rstd1[:, cs], op0=MUL, op1=MUL)

            def ln2_batch(k):
                cs = slice(k * PPC, (k + 1) * PPC)
                nc.vector.tensor_scalar_mul(out=mu2[:, cs], in0=s2[:, cs],
                                            scalar1=1.0 / N)
                nc.vector.tensor_scalar_mul(out=tv[:, cs], in0=s2q[:, cs],
                                            scalar1=1.0 / N)
                nc.vector.tensor_mul(out=ts_[:, cs], in0=mu2[:, cs],
                                     in1=mu2[:, cs])
                nc.vector.tensor_sub(out=tv[:, cs], in0=tv[:, cs],
                                     in1=ts_[:, cs])
                nc.scalar.activation(out=ts_[:, cs], in_=tv[:, cs],
                                     func=SQRT, bias=eps128[:], scale=1.0)
                nc.vector.reciprocal(out=rstd2[:, cs], in_=ts_[:, cs])
                nc.vector.scalar_tensor_tensor(
                    out=nmr2[:, cs], in0=mu2[:, cs], scalar=-1.0,
                    in1=rstd2[:, cs], op0=MUL, op1=MUL)

            def stage1_pre(P):
                """xn + pair transpose + z copy for pair P."""
                k, p = divmod(P, PPC)
                xn = act.tile([128, N], BF16, tag="xn")
                nc.scalar.activation(
                    out=xn[:], in_=x_sb[k][:, p, :], func=IDENT,
                    bias=nmr1[:, P:P + 1], scale=rstd1[:, P:P + 1])
                zxp = ps.tile([128, 2, 128], F32, tag="zxp", bufs=3)
                for blk in range(2):
                    nc.tensor.matmul(
                        zxp[:, blk, :],
                        xn[:, blk * 128:(blk + 1) * 128],
                        id128[:], start=True, stop=True)
                z_sb = act.tile([128, 2, 128], BF16, tag="z")
                nc.vector.tensor_copy(out=z_sb[:], in_=zxp[:])
                if dbg and P == 0:
                    nc.scalar.dma_start(out=xn_dbg[:], in_=xn[:])
                return z_sb

            def mix_pair(P, z_sb, wp, bp, skip_b):
                """fc1 + gelu + fc2 for both groups of pair P."""
                hpre = ps.tile([128, 2, 2, 64], F32, tag="hpre")
                if not skip_b:
                    nc.tensor.matmul(
                        hpre[:].rearrange("p a b c -> p (a b c)"),
                        bp[:, P, :], msel[:],
                        start=True, stop=False, skip_group_check=True)
                for par in range(2):
                    for mb in range(2):
                        for nb in range(2):
                            nc.tensor.matmul(
                                hpre[:, mb, par, :],
                                wp[:, par, nb, mb * 128:(mb + 1) * 128],
                                z_sb[:, nb, par * 64:(par + 1) * 64],
                                start=(skip_b and nb == 0), stop=(nb == 1),
                                skip_group_check=True)
                hs = act.tile([128, 2, 2, 64], BF16, tag="h")
                nc.scalar.activation(out=hs[:], in_=hpre[:], func=gelu_func,
                                     scale=wsi)

                tok = ps.tile([128, 256], F32, tag="tok")
                for par in range(2):
                    for mb in range(2):
                        nc.tensor.matmul(
                            tok[par * 64:(par + 1) * 64, :],
                            hs[:, mb, par, :],
                            wp[:, par, 2 + mb, :],
                            start=(mb == 0), stop=(mb == 1),
                            skip_group_check=True)
                return tok

            def stage1_post(P, tok):
                k, p = divmod(P, PPC)
                tok3 = tok.rearrange("q (d a n) -> q d a n", d=NCORE, a=2)
                x3 = x_sb[k][:, p, :].rearrange("q (d a n) -> q d a n",
                                                d=NCORE, a=2)
                nc.vector.scalar_tensor_tensor(
                    out=u_pk[k][:, :, :, p, :], in0=tok3, scalar=wsi,
                    in1=x3, op0=MUL, op1=ADD, accum_out=s2[:, P:P + 1])
                sqs = scr.tile([128, NCORE, 2, 16], BF16, tag="sqs")
                nc.scalar.activation(out=sqs[:], in_=u_pk[k][:, :, :, p, :],
                                     func=SQUARE,
                                     accum_out=s2q[:, P:P + 1])

            # ---------------- phase 1 ----------------
            for P in range(PPC + 2):
                bn1(P)
            ln1_batch(0)
            z_cur = stage1_pre(0)
            z_nxt = stage1_pre(1)
            for P in range(NPAIR):
                k, p = divmod(P, PPC)
                wp = wpool.tile([128, 2, 4, N], mmdt, tag="w")
                nc.sync.dma_start(out=wp[:], in_=wt_in[P])

                if P + PPC + 2 < NPAIR:
                    bn1(P + PPC + 2)
                if p == 1 and k + 1 < NCHUNK:
                    ln1_batch(k + 1)

                z_n2 = stage1_pre(P + 2) if P + 2 < NPAIR else None
                if dbg and P == 0:
                    nc.scalar.dma_start(out=z_dbg[:], in_=z_cur[:])
                tok = mix_pair(P, z_cur, wp,
                               b1p if not skip_b1 else None, skip_b1)
                stage1_post(P, tok)
                z_cur, z_nxt = z_nxt, z_n2

                if p == PPC - 1:
                    ln2_batch(k)
                    for pj in range(PPC):
                        Pg = k * PPC + pj
                        nc.scalar.activation(
                            out=yn_pk[k][:, :, :, pj, :],
                            in_=u_pk[k][:, :, :, pj, :], func=IDENT,
                            bias=nmr2[:, Pg:Pg + 1],
                            scale=rstd2[:, Pg:Pg + 1])
                    for t, src in ((0, yn_pk[k]), (1, u_pk[k])):
                        for par in range(2):
                            for pr2 in range(2):
                                nc.sync.dma_start(
                                    out=sendb[k // 2][:, t, pr2, :, k % 2,
                                                      par * PPC:
                                                      (par + 1) * PPC,
                                                      :].rearrange(
                                                  "d b c n -> b d (c n)"),
                                    in_=src[par * 64:(par + 1) * 64, :,
                                            pr2].rearrange(
                                                "q d c n -> q d (c n)"))
                    if k % 2 == 1:
                        h = k // 2
                        nc.gpsimd.collective_compute(
                            "AllToAll", mybir.AluOpType.bypass,
                            replica_groups=[list(range(NCORE))],
                            ins=[sendb[h].opt()], outs=[recvb[h].opt()])

            if dbg:
                nc.scalar.dma_start(out=rstd_dbg[:, 0:NPAIR], in_=rstd1[:])
                nc.scalar.dma_start(out=rstd_dbg[:, NPAIR:], in_=rstd2[:])
                nc.scalar.dma_start(out=u_dbg[:], in_=u_pk[0][:])

            # ---------------- phase 2 staging ----------------
            # [128(par2=nl//16, b), c-global 256, nlh 16]
            yn2 = const.tile([128, C, 16], BF16)
            u2 = const.tile([128, C, 16], BF16)
            for h in range(2):
                for t, dst in ((0, yn2), (1, u2)):
                    for par2 in range(2):
                        # dst c positions d*32 + (2h+kk)*8 + ci, nlh 16
                        dview = dst[par2 * 64:(par2 + 1) * 64].rearrange(
                            "q (d hh kk ci) n -> q hh d (kk ci n)",
                            d=NCORE, hh=2, kk=2)[:, h]
                        nc.sync.dma_start(
                            out=dview,
                            in_=recvb[h][:, t, par2].rearrange(
                                "d b kk c n -> b d (kk c n)"))
            y_pk = [const.tile([128, PPC, C], BF16, name=f"y{k}")
                    for k in range(NCHUNK)]
            if dbg:
                nc.scalar.dma_start(out=yn2_dbg[:], in_=yn2[:])
                nc.scalar.dma_start(out=u2_dbg[:], in_=u2[:])

            def stage2_pre(Q):
                z2p = ps.tile([128, 2, 128], F32, tag="zxp", bufs=3)
                for cb in range(2):
                    nc.tensor.matmul(
                        z2p[:, cb, :],
                        yn2[:, cb * 128:(cb + 1) * 128, Q],
                        id128[:], start=True, stop=True)
                z2 = act.tile([128, 2, 128], BF16, tag="z")
                nc.vector.tensor_copy(out=z2[:], in_=z2p[:])
                return z2

            z2_cur = stage2_pre(0)
            z2_nxt = stage2_pre(1)
            for Q in range(NPAIR):
                k, q = divmod(Q, PPC)
                cp = wpool.tile([128, 2, 4, C], mmdt, tag="w")
                nc.scalar.dma_start(out=cp[:], in_=ct_in[Q])

                z2_n2 = stage2_pre(Q + 2) if Q + 2 < NPAIR else None
                ch_ps = mix_pair(Q, z2_cur, cp,
                                 bc1p if not skip_bc1 else None, skip_bc1)
                nc.vector.scalar_tensor_tensor(
                    out=y_pk[k][:, q, :], in0=ch_ps, scalar=wsi,
                    in1=u2[:, :, Q], op0=MUL, op1=ADD)
                z2_cur, z2_nxt = z2_nxt, z2_n2
                if q == PPC - 1:
                    for par2 in range(2):
                        nc.scalar.dma_start(
                            out=ybuf[:, par2 * 16 + k * PPC:
                                     par2 * 16 + (k + 1) * PPC, :],
                            in_=y_pk[k][par2 * 64:(par2 + 1) * 64])

    nc.finalize()
    return nc


def prep_inputs(x, g1, be1, g2, be2, tw1, tb1, tw2, tb2, cw1, cb1, cw2, cb2,
                mmdt_np=ml_dtypes.bfloat16, ws=1.0):
    """Host-side sharding + weight folding. Returns in_maps for the 8 cores."""
    f = np.float32
    x = np.asarray(x, f)
    g1, be1, g2, be2 = (np.asarray(a, f) for a in (g1, be1, g2, be2))
    tw1, tb1, tw2, tb2 = (np.asarray(a, f) for a in (tw1, tb1, tw2, tb2))
    cw1, cb1, cw2, cb2 = (np.asarray(a, f) for a in (cw1, cb1, cw2, cb2))

    def wcast(a):
        a = a * ws
        if mmdt_np is not ml_dtypes.bfloat16:
            a = np.clip(a, -240.0, 240.0)
        return a.astype(mmdt_np)

    w1t = (tw1 * g1[None, None, :]).transpose(0, 2, 1)            # [C, N, M]
    bias1 = (tb1 + np.einsum('n,cmn->cm', be1, tw1)) * ws         # [C, M]
    w2t = tw2.transpose(0, 2, 1)                                  # [c, m, k]
    t1r = w1t.reshape(C, 2, 128, N)
    t2r = w2t.reshape(C, 2, 128, N)
    wt = np.ascontiguousarray(
        np.stack([t1r[:, 0], t1r[:, 1], t2r[:, 0], t2r[:, 1]],
                 axis=2))                                         # [C,128,4,N]

    c1t = (cw1 * g2[:, None, None]).transpose(0, 2, 1)            # [N, C, O]
    biasc1 = (cb1 + be2[:, None] * cw1.sum(axis=2)) * ws          # [N, O]
    c2t = cw2.transpose(0, 2, 1)                                  # [n, o, k]
    c1r = c1t.reshape(N, 2, 128, C)
    c2r = c2t.reshape(N, 2, 128, C)
    ct = np.ascontiguousarray(
        np.stack([c1r[:, 0], c1r[:, 1], c2r[:, 0], c2r[:, 1]],
                 axis=2))                                         # [N,128,4,C]

    id128 = np.eye(128, dtype=f).astype(ml_dtypes.bfloat16)
    msel = np.zeros((4, 2, 2, 64), f)
    for mb in range(2):
        for par in range(2):
            msel[mb * 2 + par, mb, par, :] = 1.0
    msel = msel.reshape(4, 256).astype(ml_dtypes.bfloat16)

    # channel pair order within a core: chunk k has channels k*8+ci,
    # pairs are (ci, ci+4); patch pairs are (nl, nl+16)
    cpair0 = np.array([k * CH + p for k in range(NCHUNK)
                       for p in range(PPC)])                      # 16
    npair0 = np.arange(16)

    def pair_pack(wfull, p0, off):   # [G,128,4,X] -> [G/2,128,2,4,X]
        a = wfull[p0]
        b = wfull[p0 + off]
        return np.ascontiguousarray(np.stack([a, b], axis=2))

    def bias_pair(bm, p0, off):      # [G,256] -> [4, G/2, 128] (mb*2+par)
        out = np.empty((4, len(p0), 128), f)
        for mb in range(2):
            for par in range(2):
                out[mb * 2 + par] = bm[p0 + par * off,
                                       mb * 128:(mb + 1) * 128]
        return np.ascontiguousarray(out).astype(ml_dtypes.bfloat16)

    in_maps = []
    for m in range(NCORE):
        cs = slice(m * CL, (m + 1) * CL)
        ns = slice(m * NL, (m + 1) * NL)
        xl = x[:, cs, :]                                          # [B,CL,N]
        # pair-packed x: [2(par), B, 16(chunk-major pairs), N]
        xp = np.stack([xl[:, cpair0, :], xl[:, cpair0 + 4, :]], axis=0)
        wtl = wcast(wt[cs])
        ctl = wcast(ct[ns])
        d = {
            "x_sh": np.ascontiguousarray(xp),
            "wt": pair_pack(wtl, cpair0, 4),
            "ct": pair_pack(ctl, npair0, 16),
            "id128": id128,
        }
        if np.any(bias1):
            d["b1p"] = bias_pair(bias1[cs], cpair0, 4)
            d["msel"] = msel
        if np.any(biasc1):
            d["bc1p"] = bias_pair(biasc1[ns], npair0, 16)
            d["msel"] = msel
        if np.any(tb2):
            d["b2r"] = (tb2[cs] * ws).astype(ml_dtypes.bfloat16)
        if np.any(cb2):
            d["bc2r"] = (cb2.T[ns] * ws).astype(ml_dtypes.bfloat16)
        in_maps.append(d)
    return in_maps


def assemble_output(results):
    """results: per-core dicts with 'ybuf' [B, NL, C] -> y [B, C, N]."""
    y = np.empty((B, C, N), np.float32)
    for k in range(NCORE):
        y[:, :, k * NL:(k + 1) * NL] = (
            results[k]["ybuf"].astype(np.float32).transpose(0, 2, 1))
    return y


_PROGRAMS = {}

# weight dtype config: (mybir dtype, numpy dtype, weight scale)
USE_FP8 = False
_W_CFG = ((mybir.dt.float8e4, ml_dtypes.float8_e4m3, 64.0) if USE_FP8
          else (BF16, ml_dtypes.bfloat16, 1.0))


def get_program(skip_b2, skip_bc2, skip_b1=True, skip_bc1=True):
    key = (skip_b1, skip_b2, skip_bc1, skip_bc2, USE_FP8)
    if key not in _PROGRAMS:
        _PROGRAMS[key] = build_program(
            mmdt=_W_CFG[0], ws=_W_CFG[2],
            skip_b1=skip_b1, skip_b2=skip_b2,
            skip_bc1=skip_bc1, skip_bc2=skip_bc2)
    return _PROGRAMS[key]


def kernel(**inputs):
    skip_b1 = not (np.any(np.asarray(inputs["tb1"]))
                   or np.any(np.asarray(inputs["be1"])))
    skip_bc1 = not (np.any(np.asarray(inputs["cb1"]))
                    or np.any(np.asarray(inputs["be2"])))
    skip_b2 = not np.any(np.asarray(inputs["tb2"]))
    skip_bc2 = not np.any(np.asarray(inputs["cb2"]))
    prog = get_program(skip_b2, skip_bc2, skip_b1, skip_bc1)
    in_maps = prep_inputs(**inputs, mmdt_np=_W_CFG[1], ws=_W_CFG[2])
    res = run_bass_kernel_spmd(prog, in_maps, list(range(NCORE)))
    return assemble_output(res.results)


if __name__ == "__main__":
    from scipy.special import erf

    rng = np.random.RandomState(0)
    s = 0.02
    inputs = dict(
        x=rng.randn(B, C, N).astype(np.float32),
        g1=np.ones(N, np.float32), be1=np.zeros(N, np.float32),
        g2=np.ones(N, np.float32), be2=np.zeros(N, np.float32),
        tw1=(rng.randn(C, N, N) * s).astype(np.float32),
        tb1=np.zeros((C, N), np.float32),
        tw2=(rng.randn(C, N, N) * s).astype(np.float32),
        tb2=np.zeros((C, N), np.float32),
        cw1=(rng.randn(N, C, C) * s).astype(np.float32),
        cb1=np.zeros((N, C), np.float32),
        cw2=(rng.randn(N, C, C) * s).astype(np.float32),
        cb2=np.zeros((N, C), np.float32),
    )

    def np_ref(x, g1, be1, g2, be2, tw1, tb1, tw2, tb2, cw1, cb1, cw2, cb2):
        def ln(z, g, b):
            mu = z.mean(-1, keepdims=True)
            var = z.var(-1, keepdims=True)
            return (z - mu) / np.sqrt(var + EPS) * g + b
        def gelu(v):
            return v * 0.5 * (1 + erf(v / np.sqrt(2.0)))
        xn = ln(x, g1, be1)
        h = gelu(np.einsum('bcn,cmn->bcm', xn, tw1) + tb1[None])
        tok = np.einsum('bcm,ckm->bck', h, tw2) + tb2[None]
        x = x + tok
        yn = ln(x, g2, be2)
        h2 = gelu(np.einsum('bcn,noc->bon', yn, cw1) + cb1.T[None])
        ch = np.einsum('bon,nko->bkn', h2, cw2) + cb2.T[None]
        return x + ch

    exp = np_ref(**{k: v.astype(np.float64) for k, v in inputs.items()})
    got = kernel(**inputs)
    err = np.abs(got - exp)
    rel = err.max() / np.abs(exp).max()
    print(f"abs err: {err.max():.3e}  rel(absmax): {rel:.3e}")
